# revision 8
# baseline (speedup 1.0000x reference)
"""AttentionCropLayer Trainium2 kernel.

Per sample b: offsets (w,h) = floor(clip(locs[b]*224, 44, 180) - 44); output
out[b] = images[b, :, w:w+88, h:h+88] * mask, with mask the fixed 88x88
sigmoid-profile outer product.

The sigmoid profile sig(10*r) - sig(10*(r-88)) is 0.5 at r=0 and within
4.6e-5 of 1.0 for r=1..87, so the mask reduces to scaling row 0 and column 0
of each crop by 0.5 (corner 0.25); the interior passes through. Max relative
error ~1e-4, far inside the 2e-2 gate.

DMA descriptors on TRN2 cost ~17.5ns fixed + bytes/21.3GBps-per-engine, so
descriptor count is the dominant cost for small transfers. The host reorders
each core's slab to channel-last [s][u][v][c]; one crop row then covers all
16 channels as a single contiguous 1408-element (5632B) descriptor: 88 read
descriptors per sample instead of 1408. The crop lands in SBUF as
[88 partitions = crop row i, free = (k, c)] and is stored in the same
channel-last layout with equally large descriptors; the host transposes the
device output [s][i][k][c] back to [s][c][i][k].

Edge scaling on device: k=0 columns are free[0:16] on every partition (one
tiny DVE op per sample); the i=0 row lives on partition 0 only, so it is
copied via one SBUF->SBUF DMA into a [16, 1408] tmp tile (partition =
sample), scaled there, and stored as the out[:, 0, :, :] rows.
"""

import sys

if "/opt/trn_rl_repo" not in sys.path:
    sys.path.insert(0, "/opt/trn_rl_repo")

import numpy as np

import concourse.bass as bass
import concourse.bacc as bacc
import concourse.mybir as mybir
from concourse import tile
from concourse.bass_utils import run_bass_kernel_spmd

TL = 44
CROP = 2 * TL          # 88
SCALE = 224.0
B, C, IN = 128, 16, 224
NCORES = 8
BPC = B // NCORES      # 16 samples per core
MAXOFF = IN - CROP     # 136
IMSZ = C * IN * IN     # elements per sample
RUN = CROP * C         # 1408: one crop row x all channels (5632B descriptor)
USTRIDE = IN * C       # 3584: element stride between consecutive u rows
SROW = CROP * RUN      # 123904: elements per sample in device-out layout
MAXEOFF = (BPC - 1) * IMSZ + C * (MAXOFF * IN + MAXOFF)

_nc_cache = {}


def _build_nc():
    nc = bacc.Bacc(None)
    images = nc.declare_dram_parameter(
        "images", [1, BPC * IMSZ], mybir.dt.float32, isOutput=False
    )
    offs = nc.declare_dram_parameter(
        "offs", [1, BPC], mybir.dt.int32, isOutput=False
    )
    # channel-last device output [s][i][k][c]; host permutes to [s][c][i][k]
    out = nc.declare_dram_parameter(
        "out", [BPC, CROP, CROP, C], mybir.dt.float32, isOutput=True
    )

    with tile.TileContext(nc) as tc:
        with (
            tc.tile_pool(name="const", bufs=1) as cpool,
            tc.tile_pool(name="work", bufs=1) as wpool,
        ):
            # warm the dynamic-DMA path on both HWDGE rings with a dummy
            # register-offset read: the first dynamic DMA per ring pays a
            # ~10us one-time cold cost (bc-ucode load); absorb it while the
            # offset staging DMA is still in flight
            regs = {}
            for rk, weng in (("sync", nc.sync), ("scalar", nc.scalar)):
                reg = weng.alloc_register(
                    "o_reg_sp" if rk == "sync" else "o_reg_act"
                )
                regs[rk] = reg
                weng.reg_mov(reg, 0)
                ov0 = weng.snap(reg, donate=True, min_val=0, max_val=0)
                wsrc = bass.AP(
                    tensor=images[:].tensor,
                    offset=ov0,
                    ap=[[64, 128], [1, 64]],
                    dep_tracking_offset=0,
                )
                wt_ = cpool.tile([128, 64], mybir.dt.float32, tag=f"warm_{rk}")
                weng.dma_start(out=wt_[:], in_=wsrc)
            offs_sb = cpool.tile([1, BPC], mybir.dt.int32)
            nc.sync.dma_start(out=offs_sb[:], in_=offs[:])

            # crop tile: partition = crop row i, free = (sample, k, c)
            t = wpool.tile([CROP, BPC * RUN], mybir.dt.float32, tag="crop")
            # partition-0 (i=0) rows regrouped as partition = sample; two
            # 8-partition tiles because compute ops must start at a
            # partition-quadrant boundary
            tmps = [
                wpool.tile([8, RUN], mybir.dt.float32, name=f"edge{g}",
                           tag=f"edge{g}")
                for g in range(2)
            ]

            engs = {"sync": nc.sync, "scalar": nc.scalar}
            for s in range(BPC):
                rk = "sync" if s % 2 == 0 else "scalar"
                eng_, reg_ = engs[rk], regs[rk]
                eng_.reg_load(reg_, offs_sb[0:1, s : s + 1])
                ov = eng_.snap(reg_, donate=True, min_val=0, max_val=MAXEOFF)
                srcap = bass.AP(
                    tensor=images[:].tensor,
                    offset=ov,
                    ap=[[USTRIDE, CROP], [1, RUN]],
                    dep_tracking_offset=s * IMSZ,
                )
                eng_.dma_start(out=t[:, s * RUN : (s + 1) * RUN], in_=srcap)
                # k=0 column (first 16 elems of the sample's run, every row)
                nc.vector.tensor_scalar_mul(
                    t[:, s * RUN : s * RUN + C],
                    t[:, s * RUN : s * RUN + C],
                    0.5,
                )
                # rows i=1..87 straight out; row 0 goes via the edge tile
                dst = bass.AP(
                    tensor=out[:].tensor,
                    offset=s * SROW + RUN,
                    ap=[[RUN, CROP - 1], [1, RUN]],
                )
                # stores: gpsimd SWDGE, except the last four go on the HWDGE
                # rings once those have issued their final reads (a store
                # waiting on compute sems must not head-block later reads)
                if s < BPC - 4:
                    seng = nc.gpsimd
                else:
                    seng = nc.sync if s % 2 == 0 else nc.scalar
                seng.dma_start(out=dst, in_=t[1:CROP, s * RUN : (s + 1) * RUN])
                if s % 8 == 7:
                    # regroup the i=0 rows of the last 8 samples: one
                    # SBUF->SBUF DMA, 8 descriptors of 5632B
                    g = s // 8
                    nc.gpsimd.dma_start(
                        out=tmps[g][:],
                        in_=t[0:1, g * 8 * RUN : (g + 1) * 8 * RUN],
                    )
                    nc.vector.tensor_scalar_mul(tmps[g][:], tmps[g][:], 0.5)
            for g in range(2):
                dst0 = bass.AP(
                    tensor=out[:].tensor,
                    offset=g * 8 * SROW,
                    ap=[[SROW, 8], [1, RUN]],
                )
                nc.gpsimd.dma_start(out=dst0, in_=tmps[g][:])
    nc.finalize()
    return nc


def _get_nc():
    if "nc" not in _nc_cache:
        _nc_cache["nc"] = _build_nc()
    return _nc_cache["nc"]


def _host_offsets(locs):
    locs = np.asarray(locs, dtype=np.float32)
    t = np.clip(locs * np.float32(SCALE), np.float32(TL), np.float32(IN - TL))
    return np.floor(t - np.float32(TL)).astype(np.int32)  # [B, 2] (w, h)


def make_in_maps(images, locs):
    images = np.asarray(images, dtype=np.float32)
    off = _host_offsets(locs)  # [B, 2] (w, h)
    s_idx = np.arange(BPC, dtype=np.int64)
    in_maps = []
    for i in range(NCORES):
        sl = slice(i * BPC, (i + 1) * BPC)
        osh = off[sl].astype(np.int64)
        eoff = (s_idx * IMSZ + C * (osh[:, 0] * IN + osh[:, 1])).astype(np.int32)
        # channel-last slab [s][u][v][c]
        slab = np.ascontiguousarray(
            np.moveaxis(images[sl], 1, -1)
        ).reshape(1, -1)
        in_maps.append(
            {
                "images": slab,
                "offs": np.ascontiguousarray(eoff.reshape(1, -1)),
            }
        )
    return in_maps


def run(images, locs, trace=False, **kwargs):
    nc = _get_nc()
    in_maps = make_in_maps(images, locs)
    res = run_bass_kernel_spmd(
        nc, in_maps, core_ids=list(range(NCORES)), trace=trace, **kwargs
    )
    outs = []
    for i in range(NCORES):
        o = np.asarray(res.results[i]["out"])  # [BPC, 88, 88, C]
        outs.append(np.moveaxis(o, -1, 1))     # -> [BPC, C, 88, 88]
    full = np.ascontiguousarray(np.concatenate(outs, axis=0)).astype(np.float32)
    return full, res


def kernel(images, locs):
    full, _ = run(images, locs, trace=False)
    return full


# revision 12
# speedup vs baseline: 1.0365x; 1.0365x over previous
"""AttentionCropLayer Trainium2 kernel.

Per sample b: offsets (w,h) = floor(clip(locs[b]*224, 44, 180) - 44); output
out[b] = images[b, :, w:w+88, h:h+88] * mask, with mask the fixed 88x88
sigmoid-profile outer product.

The sigmoid profile sig(10*r) - sig(10*(r-88)) is 0.5 at r=0 and within
4.6e-5 of 1.0 for r=1..87, so the mask reduces to scaling row 0 and column 0
of each crop by 0.5 (corner 0.25); the interior passes through. Max relative
error ~1e-4, far inside the 2e-2 gate.

DMA descriptors on TRN2 cost ~17.5ns fixed + bytes/21.3GBps-per-engine, so
descriptor count is the dominant cost for small transfers. The host reorders
each core's slab to channel-last [s][u][v][c]; one crop row then covers all
16 channels as a single contiguous 1408-element (5632B) descriptor: 88 read
descriptors per sample instead of 1408. The crop lands in SBUF as
[88 partitions = crop row i, free = (k, c)] and is stored in the same
channel-last layout with equally large descriptors; the host transposes the
device output [s][i][k][c] back to [s][c][i][k].

Edge scaling on device: k=0 columns are free[0:16] on every partition (one
tiny DVE op per sample); the i=0 row lives on partition 0 only, so it is
copied via one SBUF->SBUF DMA into a [16, 1408] tmp tile (partition =
sample), scaled there, and stored as the out[:, 0, :, :] rows.
"""

import sys

if "/opt/trn_rl_repo" not in sys.path:
    sys.path.insert(0, "/opt/trn_rl_repo")

import numpy as np

import concourse.bass as bass
import concourse.bacc as bacc
import concourse.mybir as mybir
from concourse import tile
from concourse.bass_utils import run_bass_kernel_spmd

TL = 44
CROP = 2 * TL          # 88
SCALE = 224.0
B, C, IN = 128, 16, 224
NCORES = 8
BPC = B // NCORES      # 16 samples per core
MAXOFF = IN - CROP     # 136
IMSZ = C * IN * IN     # elements per sample
RUN = CROP * C         # 1408: one crop row x all channels (5632B descriptor)
USTRIDE = IN * C       # 3584: element stride between consecutive u rows
SROW = CROP * RUN      # 123904: elements per sample in device-out layout
MAXEOFF = (BPC - 1) * IMSZ + C * (MAXOFF * IN + MAXOFF)

_nc_cache = {}


def _build_nc():
    nc = bacc.Bacc(None)
    images = nc.declare_dram_parameter(
        "images", [1, BPC * IMSZ], mybir.dt.float32, isOutput=False
    )
    offs = nc.declare_dram_parameter(
        "offs", [1, BPC], mybir.dt.int32, isOutput=False
    )
    # channel-last device output [s][i][k][c]; host permutes to [s][c][i][k]
    out = nc.declare_dram_parameter(
        "out", [BPC, CROP, CROP, C], mybir.dt.float32, isOutput=True
    )

    with tile.TileContext(nc) as tc:
        with (
            tc.tile_pool(name="const", bufs=1) as cpool,
            tc.tile_pool(name="work", bufs=1) as wpool,
        ):
            # warm the dynamic-DMA path on both HWDGE rings with a dummy
            # register-offset read: the first dynamic DMA per ring pays a
            # ~10us one-time cold cost (bc-ucode load); absorb it while the
            # offset staging DMA is still in flight
            regs = {}
            for rk, weng in (("sync", nc.sync), ("scalar", nc.scalar)):
                reg = weng.alloc_register(
                    "o_reg_sp" if rk == "sync" else "o_reg_act"
                )
                regs[rk] = reg
                weng.reg_mov(reg, 0)
                ov0 = weng.snap(reg, donate=True, min_val=0, max_val=0)
                wsrc = bass.AP(
                    tensor=images[:].tensor,
                    offset=ov0,
                    ap=[[64, 1], [1, 64]],
                    dep_tracking_offset=0,
                )
                wt_ = cpool.tile([1, 64], mybir.dt.float32, tag=f"warm_{rk}")
                weng.dma_start(out=wt_[:], in_=wsrc)
            offs_sb = cpool.tile([1, BPC], mybir.dt.int32)
            nc.sync.dma_start(out=offs_sb[:], in_=offs[:])

            # crop tile: partition = (crop row i + sample shift) mod 128,
            # free = (sample, k, c). The per-sample quadrant shift
            # sigma = 32*(s//4) spreads descriptors over all 128 partitions
            # because HWDGE descriptors stripe across DMA engines by SBUF
            # partition block; shifts stay multiples of 32 so compute ops
            # (which must start at a partition-quadrant boundary) still work.
            t = wpool.tile([128, BPC * RUN], mybir.dt.float32, tag="crop")
            # i=0 rows regrouped as partition = sample; two 8-partition
            # tiles so the scale ops start at partition 0
            tmps = [
                wpool.tile([8, RUN], mybir.dt.float32, name=f"edge{g}",
                           tag=f"edge{g}")
                for g in range(2)
            ]

            engs = {"sync": nc.sync, "scalar": nc.scalar}

            def pieces_of(sigma, r_lo):
                # rows [r_lo, 88) -> (row_start, nrows, partition_start),
                # wrapping at partition 128
                res = []
                r_split = 128 - sigma
                if r_lo < min(CROP, r_split):
                    res.append((r_lo, min(CROP, r_split) - r_lo, sigma + r_lo))
                if r_split < CROP:
                    res.append((r_split, CROP - r_split, 0))
                return res

            for s in range(BPC):
                sigma = 32 * (s // 4)
                rk = "sync" if s % 2 == 0 else "scalar"
                eng_, reg_ = engs[rk], regs[rk]
                eng_.reg_load(reg_, offs_sb[0:1, s : s + 1])
                ov = eng_.snap(reg_, donate=True, min_val=0, max_val=MAXEOFF)
                for r0, n, p0 in pieces_of(sigma, 0):
                    srcap = bass.AP(
                        tensor=images[:].tensor,
                        offset=ov + r0 * USTRIDE,
                        ap=[[USTRIDE, n], [1, RUN]],
                        dep_tracking_offset=s * IMSZ + r0 * USTRIDE,
                    )
                    eng_.dma_start(
                        out=t[p0 : p0 + n, s * RUN : (s + 1) * RUN], in_=srcap
                    )
                    # k=0 column (first 16 elems of the sample's run).
                    # Compute-op partition windows: start 0 spans up to 128,
                    # start 64 up to 64, starts 32/96 up to 32 — split to fit.
                    q0, qn = p0, n
                    while qn > 0:
                        lim = {0: 128, 32: 32, 64: 64, 96: 32}[q0]
                        step = min(qn, lim)
                        nc.vector.tensor_scalar_mul(
                            t[q0 : q0 + step, s * RUN : s * RUN + C],
                            t[q0 : q0 + step, s * RUN : s * RUN + C],
                            0.5,
                        )
                        q0 += step
                        qn -= step
                # rows i=1..87 straight out; row 0 goes via the edge tiles.
                # samples 0-11 store on gpsimd SWDGE (chunks spread evenly
                # over DMA engines); 12-15 on the rings after their reads
                seng = nc.gpsimd if s < 12 else engs[rk]
                for r0, n, p0 in pieces_of(sigma, 1):
                    dst = bass.AP(
                        tensor=out[:].tensor,
                        offset=s * SROW + r0 * RUN,
                        ap=[[RUN, n], [1, RUN]],
                    )
                    seng.dma_start(
                        out=dst, in_=t[p0 : p0 + n, s * RUN : (s + 1) * RUN]
                    )
                if s % 4 == 3:
                    # regroup i=0 rows of samples 4g..4g+3 (all on partition
                    # sigma): one SBUF->SBUF DMA, 4 descriptors of 5632B
                    g = s // 4
                    nc.gpsimd.dma_start(
                        out=tmps[g // 2][(g % 2) * 4 : (g % 2) * 4 + 4, :],
                        in_=t[sigma : sigma + 1, 4 * g * RUN : 4 * (g + 1) * RUN],
                    )
                if s == 11:
                    # groups 0+1 extracted long ago; scale their edge tile
                    # here so the op never stalls the DVE queue
                    nc.vector.tensor_scalar_mul(tmps[0][:], tmps[0][:], 0.5)
            nc.vector.tensor_scalar_mul(tmps[1][:], tmps[1][:], 0.5)
            for g, seng in ((0, nc.sync), (1, nc.scalar)):
                dst0 = bass.AP(
                    tensor=out[:].tensor,
                    offset=g * 8 * SROW,
                    ap=[[SROW, 8], [1, RUN]],
                )
                seng.dma_start(out=dst0, in_=tmps[g][:])
    nc.finalize()
    return nc


def _get_nc():
    if "nc" not in _nc_cache:
        _nc_cache["nc"] = _build_nc()
    return _nc_cache["nc"]


def _host_offsets(locs):
    locs = np.asarray(locs, dtype=np.float32)
    t = np.clip(locs * np.float32(SCALE), np.float32(TL), np.float32(IN - TL))
    return np.floor(t - np.float32(TL)).astype(np.int32)  # [B, 2] (w, h)


def make_in_maps(images, locs):
    images = np.asarray(images, dtype=np.float32)
    off = _host_offsets(locs)  # [B, 2] (w, h)
    s_idx = np.arange(BPC, dtype=np.int64)
    in_maps = []
    for i in range(NCORES):
        sl = slice(i * BPC, (i + 1) * BPC)
        osh = off[sl].astype(np.int64)
        eoff = (s_idx * IMSZ + C * (osh[:, 0] * IN + osh[:, 1])).astype(np.int32)
        # channel-last slab [s][u][v][c]
        slab = np.ascontiguousarray(
            np.moveaxis(images[sl], 1, -1)
        ).reshape(1, -1)
        in_maps.append(
            {
                "images": slab,
                "offs": np.ascontiguousarray(eoff.reshape(1, -1)),
            }
        )
    return in_maps


def run(images, locs, trace=False, **kwargs):
    nc = _get_nc()
    in_maps = make_in_maps(images, locs)
    res = run_bass_kernel_spmd(
        nc, in_maps, core_ids=list(range(NCORES)), trace=trace, **kwargs
    )
    outs = []
    for i in range(NCORES):
        o = np.asarray(res.results[i]["out"])  # [BPC, 88, 88, C]
        outs.append(np.moveaxis(o, -1, 1))     # -> [BPC, C, 88, 88]
    full = np.ascontiguousarray(np.concatenate(outs, axis=0)).astype(np.float32)
    return full, res


def kernel(images, locs):
    full, _ = run(images, locs, trace=False)
    return full
